# revision 3
# baseline (speedup 1.0000x reference)
"""DetectionLayer decode kernel v6 for Trainium2 (Bass/Tile), 8-core SPMD.

Computes, for inputs [N, 85] and anchors [N, 4] (N = 2,000,000):
    cond    = inputs[:, 5] > 0.5
    pred_yx = inputs[:, :2] * anchors[:, 2:4] + anchors[:, :2]
    pred_hw = exp(inputs[:, 2:4]) * anchors[:, 2:4]
    out     = where(cond, concat([pred_yx, pred_hw, inputs[:, 4:]]), 0)

The per-core DMA path saturates at ~360-390 GB/s regardless of queue mix,
so the only lever is HBM bytes. The rel-err budget (2e-2) is ~10x looser
than bf16 round-off (~2e-3), so the host ships inputs/anchors as bf16 and
receives the output as bf16 (upcast to f32 on host); only the score column
rides f32 so the threshold compare is bit-exact vs the f32 reference.

Rows are padded to 86 columns so each row is 43 aligned int32 words: the
row-mask apply (the big per-element op) runs as int32 bitwise-AND against
a 0/~0 mask, halving DVE element count (a broadcast operand blocks the
DVE 2x bf16 perf mode, so plain bf16 multiply would run at 1x).

Device bytes/core: ~46.8 MB read + 43.3 MB write (vs 89.7 + 85.7 in f32).

Sharding: row dimension split into 8 equal-shape overlapping windows
(window R rows, stride S; 7*S + R == N) so every core runs the same NEFF
on a 128*K-row-aligned shard with no host-side padding copies.
"""
import sys

sys.path.insert(0, "/opt/trn_rl_repo")

import numpy as np
import ml_dtypes

import concourse.bacc as bacc
import concourse.mybir as mybir
from concourse.bass_utils import run_bass_kernel_spmd
from concourse.tile import TileContext

N = 2_000_000
C = 85
C2 = 86            # padded row length (43 int32 words)
W = C2 // 2        # int32 words per row
N_CORES = 8
P = 128            # SBUF partitions
K = 82             # anchor rows per partition per tile
TILE_ROWS = P * K  # 10496
T = 24             # tiles per core
R = T * TILE_ROWS  # 251,904 rows per core window
S = 249_728        # window stride; 7*S + R == N
THR = 0.5
BF16 = mybir.dt.bfloat16
I32 = mybir.dt.int32
F32 = mybir.dt.float32
NP_BF16 = ml_dtypes.bfloat16

assert 7 * S + R == N and S % P == 0 and S <= R

_NC_CACHE = None


def _build_module(n_tiles=T):
    rows = n_tiles * TILE_ROWS
    nc = bacc.Bacc("TRN2", target_bir_lowering=False, debug=False)
    inp = nc.dram_tensor("inputs", [rows, C2], BF16, kind="ExternalInput")
    anc = nc.dram_tensor("anchors", [rows, 4], BF16, kind="ExternalInput")
    sco = nc.dram_tensor("scores", [rows, 1], F32, kind="ExternalInput")
    out = nc.dram_tensor("out", [rows, C2], BF16, kind="ExternalOutput")

    # Slab mapping: partition p owns rows [p*nt*K, (p+1)*nt*K); within the
    # slab, tile t covers rows t*K..(t+1)*K. Row-group chunks are 14104B
    # contiguous per partition (82 rows x 172B).
    iv = inp.ap().rearrange("(p t g) c -> t p (g c)", p=P, g=K)  # [nt, 128, K*C2]
    ov = out.ap().rearrange("(p t g) c -> t p (g c)", p=P, g=K)
    av_all = anc.ap().rearrange("(p t g) c -> p (t g c)", p=P, g=K)
    sv_all = sco.ap().rearrange("(p t g) c -> p (t g c)", p=P, g=K)

    with TileContext(nc) as tc:
        with tc.tile_pool(name="anc", bufs=1) as apool, \
             tc.tile_pool(name="inp", bufs=5) as ipool, \
             tc.tile_pool(name="outp", bufs=4) as opool, \
             tc.tile_pool(name="amp", bufs=3) as mpool:
            anc_all = apool.tile([P, n_tiles * K * 4], BF16, tag="anc_all")
            sco_all = apool.tile([P, n_tiles * K], F32, tag="sco_all")
            mskf_all = apool.tile([P, n_tiles * K], F32, tag="mskf_all")
            mski_all = apool.tile([P, n_tiles * K], I32, tag="mski_all")
            # Preloads: anchors + scores are fully resident (~23 KB/part).
            nc.scalar.dma_start(out=sco_all[:], in_=sv_all)
            nc.scalar.dma_start(out=anc_all[:], in_=av_all)
            # Row masks for the whole window, in two encodings:
            # f32 1/0 (exact compare, also feeds the am multiply) and
            # int32 0/~0 (passthrough AND; -1.0 -> int32 gives all-ones).
            nc.vector.tensor_single_scalar(
                mskf_all[:], sco_all[:], THR, mybir.AluOpType.is_gt
            )
            nc.vector.tensor_scalar_mul(mski_all[:], mskf_all[:], -1.0)
            for t in range(n_tiles):
                in_t = ipool.tile([P, K * C2], BF16, tag="in")
                out_t = opool.tile([P, K * C2], BF16, tag="out")
                am_t = mpool.tile([P, K * 4], BF16, tag="am")

                nc.sync.dma_start(out=in_t[:], in_=iv[t])

                ing = in_t[:].rearrange("p (g c) -> p g c", c=C2)
                outg = out_t[:].rearrange("p (g c) -> p g c", c=C2)
                inw = in_t[:].bitcast(I32).rearrange("p (g c) -> p g c", c=W)
                outw = out_t[:].bitcast(I32).rearrange("p (g c) -> p g c", c=W)
                ang = anc_all[:, t * K * 4:(t + 1) * K * 4].rearrange(
                    "p (g c) -> p g c", c=4)
                amg = am_t[:].rearrange("p (g c) -> p g c", c=4)
                mbg = mskf_all[:, t * K:(t + 1) * K].rearrange(
                    "p (g c) -> p g c", c=1)
                mig = mski_all[:, t * K:(t + 1) * K].rearrange(
                    "p (g c) -> p g c", c=1)

                # out = mask & in (row-masked copy; cols 0..3 redone below)
                nc.vector.tensor_tensor(
                    outw, mig.broadcast_to([P, K, W]), inw,
                    mybir.AluOpType.bitwise_and)
                # masked anchors: am = mask * anchors
                nc.vector.tensor_mul(amg, mbg.broadcast_to([P, K, 4]), ang)
                # in[:, 2:4] = exp(in[:, 2:4]) in place on the scalar engine
                nc.scalar.activation(
                    ing[:, :, 2:4],
                    ing[:, :, 2:4],
                    mybir.ActivationFunctionType.Exp,
                )
                # out[:, 0:4] = [in_yx, exp(in_hw)] * [am_hw, am_hw]
                nc.vector.tensor_mul(
                    outg[:, :, 0:4].rearrange("p g (a b) -> p g a b", b=2),
                    ing[:, :, 0:4].rearrange("p g (a b) -> p g a b", b=2),
                    amg[:, :, 2:4].unsqueeze(2).broadcast_to([P, K, 2, 2]),
                )
                # out[:, 0:2] += am_yx
                nc.vector.tensor_add(outg[:, :, 0:2], outg[:, :, 0:2], amg[:, :, 0:2])

                nc.gpsimd.dma_start(out=ov[t], in_=out_t[:])
    nc.compile()
    return nc


def _get_module():
    global _NC_CACHE
    if _NC_CACHE is None:
        _NC_CACHE = _build_module()
    return _NC_CACHE


def _run(inputs, anchors, **spmd_kwargs):
    inputs = np.asarray(inputs, dtype=np.float32)
    anchors = np.asarray(anchors, dtype=np.float32)
    assert inputs.shape == (N, C) and anchors.shape == (N, 4)

    scores = np.ascontiguousarray(inputs[:, 5:6])          # exact f32 scores
    inputs_bf = np.zeros((N, C2), dtype=NP_BF16)
    inputs_bf[:, :C] = inputs                              # cast-assign to bf16
    anchors_bf = anchors.astype(NP_BF16)

    nc = _get_module()
    in_maps = [
        {
            "inputs": inputs_bf[i * S : i * S + R],
            "anchors": anchors_bf[i * S : i * S + R],
            "scores": scores[i * S : i * S + R],
        }
        for i in range(N_CORES)
    ]
    res = run_bass_kernel_spmd(nc, in_maps, core_ids=list(range(N_CORES)), **spmd_kwargs)

    out = np.empty((N, C), dtype=np.float32)
    for i in range(N_CORES - 1):
        out[i * S : (i + 1) * S] = res.results[i]["out"][:S, :C]
    out[(N_CORES - 1) * S :] = res.results[N_CORES - 1]["out"][:, :C]
    return out, res


def kernel(inputs, anchors):
    out, _ = _run(inputs, anchors)
    return out


if __name__ == "__main__":
    rng = np.random.default_rng(0)
    x = rng.random((N, C), dtype=np.float32)
    a = rng.random((N, 4), dtype=np.float32)
    y = kernel(x, a)
    print("ran ok", y.shape, y.dtype)

